# revision 7
# baseline (speedup 1.0000x reference)
"""Trainium2 Bass kernel for causal self-attention with LoRA on q/v.

Reference shapes: hidden_states [4, 2048, 1024], 16 heads x 64 dims,
LoRA rank 8 (scale 2.0) on q and v projections.

Sharding: 8 cores = 4 batches x 2 head-groups. Core c handles batch
c//2 and heads (c%2)*8 .. (c%2)*8+8. Outputs are disjoint; assembled
host-side, no collectives.

Host-side prep folds the rank-8 LoRA update into the dense weights
(W' = W + scale*B@A, exact in fp32) and the v bias into a final output
add (softmax weights sum to 1, so sum_s p_s (v_s+bv) / sum_s p_s =
out + bv).

Per-core kernel (bf16 matmuls, fp32 accumulation):
  - q/k projections in [dh-chunk, t] orientation; bias via the
    epilogue's per-partition tensor_scalar add (GpSimd).
  - v projection in [t-chunk, ch] orientation; epilogue scatters into a
    [s-chunk, 16, head, 65] buffer whose last column is constant 1.0
    (carries the softmax denominator through the PV matmul).
  - scores^T per head: K=64 matmuls (lhsT = k chunk, rhs = q tile, both
    sliced to the head's 64 partitions) into packed PSUM groups; one
    Exp activation per group (scale=1/8) writes bf16 into a per-head
    flat E buffer; diagonal 128x128 blocks get a triangular mask
    multiply on DVE.
  - PV: po[t-block, 0:65] += E_chunk.T @ [v | 1]; column 64 accumulates
    the denominator. DVE reciprocal + scale, DMA out per (head, block).

Score blocks are packed into 34 full 512-col bank slots per head with
zero padding: full 512-wide blocks are bank-aligned; the 12 partial
diagonal blocks (384/256/128) pair into slots (384+128, 256+256)
emitted at the later partner's position. Slots group into 14
activations per head on two alternating PSUM tiles (1536/1024).

PSUM: scA 3 + scB 2 + proj 1 + pv 2 = 8 banks.  A start=True matmul
zeroes its whole PSUM bank, so PV accumulators use two bank-exclusive
tiles (ring on m mod 2), each fully accumulated + read before its bank
is reused.
"""

import sys

if "/opt/trn_rl_repo" not in sys.path:
    sys.path.insert(0, "/opt/trn_rl_repo")

import numpy as np
import ml_dtypes

BF16 = ml_dtypes.bfloat16

B, T, H, NH, DH = 4, 2048, 1024, 16, 64
N_CORES = 8
HPC = 8
CH = HPC * DH
LORA_SCALE = 2.0

_cached = {}

GROUP_SIZES = [3, 2, 3, 2, 3, 2, 3, 2, 3, 2, 3, 2, 3, 1]   # slots per group
N_GROUPS = len(GROUP_SIZES)


def _head_layout():
    """Packed per-head score layout and PV emission plan.

    Returns:
      groups: list (len 14) of lists of blocks (sb, c, t0, w, off) where
        off is the column offset within the group's PSUM tile.
      block_pos: (sb, c) -> (group_idx, flat E column offset)
      pv_plan: list (len 15) of batches; batch g is emitted after
        score_group(g-1) (g in 1..13) or after all groups (g=14). Each
        entry is (m, s2, first, last).
    """
    slots = []
    held = {}
    for sb in range(16):
        t0 = sb * 128
        c0 = t0 // 512
        r = t0 - c0 * 512
        if r != 0:
            held[(sb, 512 - r)] = (sb, c0, t0, 512 - r)
            fc = range(c0 + 1, 4)
        else:
            fc = range(c0, 4)
        for c in fc:
            slots.append([(sb, c, c * 512, 512, 0)])
        if sb % 4 == 3:
            slots.append([held.pop((sb - 2, 384)) + (0,),
                          held.pop((sb, 128)) + (384,)])
        if sb % 8 == 6:
            slots.append([held.pop((sb - 4, 256)) + (0,),
                          held.pop((sb, 256)) + (256,)])
    assert not held and len(slots) == 34

    groups = []
    block_pos = {}
    si = 0
    ecol = 0
    for gi, gs in enumerate(GROUP_SIZES):
        blocks = []
        for k in range(gs):
            for (sb, c, t0, w, o) in slots[si + k]:
                blocks.append((sb, c, t0, w, k * 512 + o))
                block_pos[(sb, c)] = (gi, ecol + k * 512 + o)
        si += gs
        ecol += 512 * gs
        groups.append(blocks)
    assert si == 34 and ecol == 34 * 512

    blk_group = {k: v[0] for k, v in block_pos.items()}
    ready = {}
    for m in range(16):
        for s2 in range(m + 1):
            ready[(m, s2)] = blk_group[(s2, m // 4)] + 1
    # 2-bank PV ring: a start=True zeroes its whole PSUM bank, so the
    # bank's previous accumulator (m-2) must be fully read first; +1 group
    # of slack keeps the Tensor queue from waiting on the DVE epilogue.
    start, epi = {}, {}
    for m in range(16):
        s = max(ready[(m, 0)], 1)
        if m >= 2:
            s = max(s, min(epi[m - 2] + 1, N_GROUPS))
        start[m] = s
        epi[m] = max(max(ready[(m, s2)] for s2 in range(m + 1)), s)
    for m in range(2, 16):
        assert start[m] >= epi[m - 2]
    pv_plan = [[] for _ in range(N_GROUPS + 1)]
    for m in range(16):
        order = sorted(range(m + 1),
                       key=lambda s2: (max(ready[(m, s2)], start[m]), s2))
        for i, s2 in enumerate(order):
            g = min(max(ready[(m, s2)], start[m]), N_GROUPS)
            pv_plan[g].append((m, s2, i == 0, i == m))
    # within each batch, ascending m keeps slot handoff (epi(m-7) before
    # start(m)) in emission order
    for g in range(N_GROUPS + 1):
        pv_plan[g].sort(key=lambda t: (t[0], t[1]))

    # v-piece deadlines: v_sb[s2] must be emitted before the first batch
    # that reads it
    v_deadline = {}
    for g in range(1, N_GROUPS + 1):
        for (m, s2, f, l) in pv_plan[g]:
            v_deadline.setdefault(s2, g)
    return groups, block_pos, pv_plan, v_deadline


def _build_nc(masknz):
    import concourse.mybir as mybir
    from concourse import bacc
    from concourse.tile import TileContext

    dt = mybir.dt
    AF = mybir.ActivationFunctionType

    nc = bacc.Bacc()

    xT_d = nc.dram_tensor("xT", [4, 128, 8, 512], dt.bfloat16, kind="ExternalInput")
    wqT_d = nc.dram_tensor("wqT", [128, 8, 512], dt.bfloat16, kind="ExternalInput")
    wkT_d = nc.dram_tensor("wkT", [128, 8, 512], dt.bfloat16, kind="ExternalInput")
    wvT_d = nc.dram_tensor("wvT", [128, 8, 512], dt.bfloat16, kind="ExternalInput")
    bqk_d = nc.dram_tensor("bqk", [128, 2, 4], dt.float32, kind="ExternalInput")
    tri_d = nc.dram_tensor("tri", [128, 128], dt.bfloat16, kind="ExternalInput")
    if masknz:
        emask_d = nc.dram_tensor("emask", [128, 16], dt.float32, kind="ExternalInput")
    out_d = nc.dram_tensor("out", [T, CH], dt.float32, kind="ExternalOutput")

    GROUPS, BLOCK_POS, PV_PLAN, V_DEADLINE = _head_layout()

    with TileContext(nc) as tc:
        with (
            tc.tile_pool(name="const", bufs=1) as cpool,
            tc.tile_pool(name="big", bufs=1) as bpool,
            tc.tile_pool(name="small", bufs=8) as spool,
            tc.tile_pool(name="psproj", bufs=1, space="PSUM") as ps_proj,
            tc.tile_pool(name="pssc", bufs=1, space="PSUM") as ps_sc,
            tc.tile_pool(name="pspv", bufs=1, space="PSUM") as ps_pv,
        ):
            tri_sb = cpool.tile([128, 128], dt.bfloat16, tag="tri")
            nc.sync.dma_start(tri_sb[:], tri_d[:])
            bqk_sb = cpool.tile([128, 2, 4], dt.float32, tag="bqk")
            nc.sync.dma_start(bqk_sb[:], bqk_d[:])
            if masknz:
                emask_sb = cpool.tile([128, 16], dt.float32, tag="emask")
                nc.sync.dma_start(emask_sb[:], emask_d[:])

            x_sb = [None] * 4

            def load_x(tb):
                xt = bpool.tile([128, 8, 512], dt.bfloat16, tag=f"x{tb}",
                                name=f"x{tb}")
                nc.sync.dma_start(xt[:], xT_d[tb])
                x_sb[tb] = xt

            load_x(0)
            wq_sb = bpool.tile([128, 8, 512], dt.bfloat16, tag="wq")
            nc.sync.dma_start(wq_sb[:], wqT_d[:])
            wk_sb = bpool.tile([128, 8, 512], dt.bfloat16, tag="wk")
            nc.sync.dma_start(wk_sb[:], wkT_d[:])
            wv_sb = bpool.tile([128, 8, 512], dt.bfloat16, tag="wv")
            nc.sync.dma_start(wv_sb[:], wvT_d[:])
            for tb in range(1, 4):
                load_x(tb)

            qt = [[bpool.tile([128, 512], dt.bfloat16, tag=f"q{j}_{tb}",
                              name=f"q{j}_{tb}") for tb in range(4)]
                  for j in range(4)]
            kt = [[bpool.tile([128, 512], dt.bfloat16, tag=f"k{j}_{tb}",
                              name=f"k{j}_{tb}") for tb in range(4)]
                  for j in range(4)]
            v_sb = bpool.tile([128, 16, 8, 65], dt.bfloat16, tag="v")
            nc.gpsimd.memset(v_sb[:, :, :, 64:65], 1.0)
            e_sb = [bpool.tile([128, 34 * 512], dt.bfloat16, tag=f"e{s}",
                               name=f"e{s}") for s in range(2)]

            # ---- projections ------------------------------------------
            def proj_qk(j, tb, which):
                ms = slice(j * 128, (j + 1) * 128)
                w = wq_sb if which == 0 else wk_sb
                dst = (qt if which == 0 else kt)[j][tb]
                p = ps_proj.tile([128, 512], dt.float32, tag="proj", name="pqk")
                for kc in range(8):
                    nc.tensor.matmul(p[:], w[:, kc, ms], x_sb[tb][:, kc, :],
                                     start=(kc == 0), stop=(kc == 7))
                nc.vector.tensor_scalar_add(dst[:], p[:],
                                            bqk_sb[:, which, j:j + 1])

            def proj_v(m):
                p = ps_proj.tile([128, 512], dt.float32, tag="proj", name="pv")
                msl = slice((m % 4) * 128, (m % 4 + 1) * 128)
                for kc in range(8):
                    nc.tensor.matmul(p[:], x_sb[m // 4][:, kc, msl],
                                     wv_sb[:, kc, :],
                                     start=(kc == 0), stop=(kc == 7))
                nc.vector.tensor_copy(
                    v_sb[:, m, :, 0:64],
                    p[:].rearrange("p (h d) -> p h d", h=8))

            # ---- attention --------------------------------------------
            def score_group(h, gi):
                j, p = h // 2, h % 2
                psl = slice(p * 64, p * 64 + 64)
                blocks = GROUPS[gi]
                ncols = 512 * GROUP_SIZES[gi]
                width = 1536 if gi % 2 == 0 else 1024
                sc = ps_sc.tile([128, width], dt.float32,
                                tag=f"sc{gi % 2}", name=f"sc{gi % 2}")
                for (sb, c, t0, w, off) in blocks:
                    lhs = kt[j][sb // 4][psl,
                                         (sb % 4) * 128:(sb % 4) * 128 + 128]
                    rhs = qt[j][c][psl, t0 - c * 512: t0 - c * 512 + w]
                    nc.tensor.matmul(sc[:, off:off + w], lhs, rhs,
                                     start=True, stop=True)
                eoff = sum(GROUP_SIZES[:gi]) * 512
                nc.scalar.activation(e_sb[h % 2][:, eoff:eoff + ncols],
                                     sc[:, 0:ncols], AF.Exp, scale=0.125)
                for (sb, c, t0, w, off) in blocks:
                    boff = eoff + off
                    if masknz:
                        nc.gpsimd.tensor_scalar_mul(
                            e_sb[h % 2][:, boff:boff + w],
                            e_sb[h % 2][:, boff:boff + w],
                            emask_sb[:, sb:sb + 1])
                    if c == sb // 4:        # diagonal block: tri mask
                        nc.gpsimd.tensor_mul(
                            e_sb[h % 2][:, boff:boff + 128],
                            e_sb[h % 2][:, boff:boff + 128], tri_sb[:])

            def pv_batch(h, g, po):
                for (m, s2, first, last) in PV_PLAN[g]:
                    c = m // 4
                    gi_b, ecol = BLOCK_POS[(s2, c)]
                    t0 = max(c * 512, s2 * 128)
                    off = ecol + m * 128 - t0
                    pm = po[m % 2]
                    nc.tensor.matmul(pm[:],
                                     e_sb[h % 2][:, off:off + 128],
                                     v_sb[:, s2, h, :],
                                     start=first, stop=last,
                                     skip_group_check=True)
                    if last:
                        rz = spool.tile([128, 1], dt.float32, tag="rz",
                                        name="rz")
                        nc.vector.reciprocal(rz[:], pm[:, 64:65])
                        ot = spool.tile([128, 64], dt.float32, tag="ot",
                                        name="ot")
                        nc.vector.tensor_scalar_mul(ot[:], pm[:, 0:64],
                                                    rz[:])
                        nc.sync.dma_start(
                            out_d[m * 128:(m + 1) * 128,
                                  h * 64:(h + 1) * 64], ot[:])

            # ---- global emission schedule -----------------------------
            # fillers: (deadline (head, group) or None, closure)
            fillers = []

            def F(fn, dl=None):
                fillers.append((dl, fn))

            # head-0 k pieces: kt[0][tb] first used at sb=4tb
            first_group_of_sb = {}
            for gi, blocks in enumerate(GROUPS):
                for (sb, c, t0, w, off) in blocks:
                    first_group_of_sb.setdefault(sb, gi)
            F(lambda: proj_qk(0, 3, 0), (0, 1))             # qt[0][3] at c3
            for tb in (1, 2, 3):
                F(lambda tb=tb: proj_qk(0, tb, 1),
                  (0, first_group_of_sb[4 * tb]))
            for m in range(16):
                F(lambda m=m: proj_v(m),
                  (0, min(V_DEADLINE.get(m, N_GROUPS) - 1, N_GROUPS - 1)))
            for j in (1, 2, 3):
                for which in (0, 1):
                    for tb in range(4):
                        F(lambda j=j, tb=tb, w=which: proj_qk(j, tb, w),
                          (2 * j, 0))

            def drain_fillers(now, budget):
                done = 0
                rest = []
                for (dl, fn) in fillers:
                    if dl is not None and dl <= now:
                        fn()
                        done += 1
                    else:
                        rest.append((dl, fn))
                fillers[:] = rest
                while done < budget and fillers:
                    _, fn = fillers.pop(0)
                    fn()
                    done += 1

            # startup: minimum pieces for head-0 group 0 (sb0: c0,c1,c2)
            proj_qk(0, 0, 0)
            proj_qk(0, 1, 0)
            proj_qk(0, 2, 0)
            proj_qk(0, 0, 1)

            for h in range(8):
                po = [ps_pv.tile([128, 65], dt.float32, tag=f"po{s}",
                                 name=f"po{s}") for s in range(2)]
                for gi in range(N_GROUPS):
                    drain_fillers((h, gi), 2 if h < 2 else 1)
                    score_group(h, gi)
                    if gi + 1 <= N_GROUPS - 1:
                        pv_batch(h, gi + 1, po)
                pv_batch(h, N_GROUPS, po)
            assert not fillers, f"{len(fillers)} fillers left"

    nc.compile()
    return nc


def _prep_core_inputs(c, x, mask, WqF, bq, Wk, bk, WvF):
    b, half = divmod(c, 2)
    hs = half * CH

    xT = np.ascontiguousarray(x[b].T.astype(BF16))  # [1024, 2048]
    xTd = np.ascontiguousarray(xT.reshape(8, 128, 4, 512).transpose(2, 1, 0, 3))

    def wT(W):
        Ws = W[hs:hs + CH]
        return np.ascontiguousarray(
            Ws.T.astype(BF16).reshape(8, 128, 512).transpose(1, 0, 2))

    bqk = np.ascontiguousarray(
        np.stack([bq[hs:hs + CH].reshape(4, 128).T,
                  bk[hs:hs + CH].reshape(4, 128).T], axis=1).astype(np.float32))

    tri = np.triu(np.ones((128, 128), BF16))

    d = {"xT": xTd, "wqT": wT(WqF), "wkT": wT(Wk), "wvT": wT(WvF),
         "bqk": bqk, "tri": tri}
    if (mask != 0).any():
        em = np.exp(mask[b, 0, 0]).reshape(16, 128).T.astype(np.float32)
        d["emask"] = np.ascontiguousarray(em)
    return d


def _run(inputs, trace=False, trace_kwargs=None):
    from concourse.bass_utils import run_bass_kernel_spmd

    args = {k: np.asarray(v) for k, v in inputs.items()}
    WqF = (args["Wq"].astype(np.float64)
           + LORA_SCALE * (args["qB"].astype(np.float64)
                           @ args["qA"].astype(np.float64))).astype(np.float32)
    WvF = (args["Wv"].astype(np.float64)
           + LORA_SCALE * (args["vB"].astype(np.float64)
                           @ args["vA"].astype(np.float64))).astype(np.float32)
    mask = args["attention_mask"]
    masknz = bool((mask != 0).any())

    in_maps = [
        _prep_core_inputs(c, args["hidden_states"], mask,
                          WqF, args["bq"], args["Wk"], args["bk"], WvF)
        for c in range(N_CORES)
    ]

    key = f"nc{int(masknz)}"
    if key not in _cached:
        _cached[key] = _build_nc(masknz)
    nc = _cached[key]

    res = run_bass_kernel_spmd(
        nc, in_maps, core_ids=list(range(N_CORES)), trace=trace,
        **(trace_kwargs or {}),
    )

    bv = args["bv"]
    full = np.empty((B, T, H), np.float32)
    for c in range(N_CORES):
        b, half = divmod(c, 2)
        hs = half * CH
        full[b, :, hs:hs + CH] = res.results[c]["out"] + bv[hs:hs + CH]
    return full, res


def kernel(**inputs):
    full, _ = _run(inputs, trace=False)
    return full


# revision 8
# speedup vs baseline: 1.0390x; 1.0390x over previous
"""Trainium2 Bass kernel for causal self-attention with LoRA on q/v.

Reference shapes: hidden_states [4, 2048, 1024], 16 heads x 64 dims,
LoRA rank 8 (scale 2.0) on q and v projections.

Sharding: 8 cores = 4 batches x 2 head-groups. Core c handles batch
c//2 and heads (c%2)*8 .. (c%2)*8+8. Outputs are disjoint; assembled
host-side, no collectives.

Host-side prep folds the rank-8 LoRA update into the dense weights
(W' = W + scale*B@A, exact in fp32) and the v bias into a final output
add (softmax weights sum to 1, so sum_s p_s (v_s+bv) / sum_s p_s =
out + bv).

Per-core kernel (bf16 matmuls, fp32 accumulation):
  - q/k projections in [dh-chunk, t] orientation; bias via the
    epilogue's per-partition tensor_scalar add (GpSimd).
  - v projection in [t-chunk, ch] orientation; epilogue scatters into a
    [s-chunk, 16, head, 65] buffer whose last column is constant 1.0
    (carries the softmax denominator through the PV matmul).
  - scores^T per head: K=64 matmuls (lhsT = k chunk, rhs = q tile, both
    sliced to the head's 64 partitions) into packed PSUM groups; one
    Exp activation per group (scale=1/8) writes bf16 into a per-head
    flat E buffer; diagonal 128x128 blocks get a triangular mask
    multiply on DVE.
  - PV: po[t-block, 0:65] += E_chunk.T @ [v | 1]; column 64 accumulates
    the denominator. DVE reciprocal + scale, DMA out per (head, block).

Score blocks are packed into 34 full 512-col bank slots per head with
zero padding: full 512-wide blocks are bank-aligned; the 12 partial
diagonal blocks (384/256/128) pair into slots (384+128, 256+256)
emitted at the later partner's position. Slots group into 14
activations per head on two alternating PSUM tiles (1536/1024).

PSUM: scA 3 + scB 2 + proj 1 + pv 2 = 8 banks.  A start=True matmul
zeroes its whole PSUM bank, so PV accumulators use two bank-exclusive
tiles (ring on m mod 2), each fully accumulated + read before its bank
is reused.
"""

import sys

if "/opt/trn_rl_repo" not in sys.path:
    sys.path.insert(0, "/opt/trn_rl_repo")

import numpy as np
import ml_dtypes

BF16 = ml_dtypes.bfloat16

B, T, H, NH, DH = 4, 2048, 1024, 16, 64
N_CORES = 8
HPC = 8
CH = HPC * DH
LORA_SCALE = 2.0

_cached = {}

GROUP_SIZES = [3, 2, 3, 2, 3, 2, 3, 2, 3, 2, 3, 2, 3, 1]   # slots per group
N_GROUPS = len(GROUP_SIZES)


def _head_layout():
    """Packed per-head score layout and PV emission plan.

    Returns:
      groups: list (len 14) of lists of blocks (sb, c, t0, w, off) where
        off is the column offset within the group's PSUM tile.
      block_pos: (sb, c) -> (group_idx, flat E column offset)
      pv_plan: list (len 15) of batches; batch g is emitted after
        score_group(g-1) (g in 1..13) or after all groups (g=14). Each
        entry is (m, s2, first, last).
    """
    slots = []
    held = {}
    for sb in range(16):
        t0 = sb * 128
        c0 = t0 // 512
        r = t0 - c0 * 512
        if r != 0:
            held[(sb, 512 - r)] = (sb, c0, t0, 512 - r)
            fc = range(c0 + 1, 4)
        else:
            fc = range(c0, 4)
        for c in fc:
            slots.append([(sb, c, c * 512, 512, 0)])
        if sb % 4 == 3:
            slots.append([held.pop((sb - 2, 384)) + (0,),
                          held.pop((sb, 128)) + (384,)])
        if sb % 8 == 6:
            slots.append([held.pop((sb - 4, 256)) + (0,),
                          held.pop((sb, 256)) + (256,)])
    assert not held and len(slots) == 34

    groups = []
    block_pos = {}
    si = 0
    ecol = 0
    for gi, gs in enumerate(GROUP_SIZES):
        blocks = []
        for k in range(gs):
            for (sb, c, t0, w, o) in slots[si + k]:
                blocks.append((sb, c, t0, w, k * 512 + o))
                block_pos[(sb, c)] = (gi, ecol + k * 512 + o)
        si += gs
        ecol += 512 * gs
        groups.append(blocks)
    assert si == 34 and ecol == 34 * 512

    blk_group = {k: v[0] for k, v in block_pos.items()}
    ready = {}
    for m in range(16):
        for s2 in range(m + 1):
            ready[(m, s2)] = blk_group[(s2, m // 4)] + 1
    # 2-bank PV ring: a start=True zeroes its whole PSUM bank, so the
    # bank's previous accumulator (m-2) must be fully read first; +1 group
    # of slack keeps the Tensor queue from waiting on the DVE epilogue.
    start, epi = {}, {}
    for m in range(16):
        s = max(ready[(m, 0)], 1)
        if m >= 2:
            s = max(s, min(epi[m - 2] + 1, N_GROUPS))
        start[m] = s
        epi[m] = max(max(ready[(m, s2)] for s2 in range(m + 1)), s)
    for m in range(2, 16):
        assert start[m] >= epi[m - 2]
    pv_plan = [[] for _ in range(N_GROUPS + 1)]
    for m in range(16):
        order = sorted(range(m + 1),
                       key=lambda s2: (max(ready[(m, s2)], start[m]), s2))
        for i, s2 in enumerate(order):
            g = min(max(ready[(m, s2)], start[m]), N_GROUPS)
            pv_plan[g].append((m, s2, i == 0, i == m))
    # within each batch, ascending m keeps slot handoff (epi(m-7) before
    # start(m)) in emission order
    for g in range(N_GROUPS + 1):
        pv_plan[g].sort(key=lambda t: (t[0], t[1]))

    # v-piece deadlines: v_sb[s2] must be emitted before the first batch
    # that reads it
    v_deadline = {}
    for g in range(1, N_GROUPS + 1):
        for (m, s2, f, l) in pv_plan[g]:
            v_deadline.setdefault(s2, g)
    return groups, block_pos, pv_plan, v_deadline


def _build_nc(masknz):
    import concourse.mybir as mybir
    from concourse import bacc
    from concourse.tile import TileContext

    dt = mybir.dt
    AF = mybir.ActivationFunctionType

    nc = bacc.Bacc()

    xT_d = nc.dram_tensor("xT", [4, 128, 8, 512], dt.bfloat16, kind="ExternalInput")
    wqT_d = nc.dram_tensor("wqT", [128, 8, 512], dt.bfloat16, kind="ExternalInput")
    wkT_d = nc.dram_tensor("wkT", [128, 8, 512], dt.bfloat16, kind="ExternalInput")
    wvT_d = nc.dram_tensor("wvT", [128, 8, 512], dt.bfloat16, kind="ExternalInput")
    bqk_d = nc.dram_tensor("bqk", [128, 2, 4], dt.float32, kind="ExternalInput")
    tri_d = nc.dram_tensor("tri", [128, 128], dt.bfloat16, kind="ExternalInput")
    if masknz:
        emask_d = nc.dram_tensor("emask", [128, 16], dt.float32, kind="ExternalInput")
    out_d = nc.dram_tensor("out", [T, CH], dt.float32, kind="ExternalOutput")

    GROUPS, BLOCK_POS, PV_PLAN, V_DEADLINE = _head_layout()

    with TileContext(nc) as tc:
        with (
            tc.tile_pool(name="const", bufs=1) as cpool,
            tc.tile_pool(name="big", bufs=1) as bpool,
            tc.tile_pool(name="small", bufs=8) as spool,
            tc.tile_pool(name="psproj", bufs=1, space="PSUM") as ps_proj,
            tc.tile_pool(name="pssc", bufs=1, space="PSUM") as ps_sc,
            tc.tile_pool(name="pspv", bufs=1, space="PSUM") as ps_pv,
        ):
            tri_sb = cpool.tile([128, 128], dt.bfloat16, tag="tri")
            nc.sync.dma_start(tri_sb[:], tri_d[:])
            bqk_sb = cpool.tile([128, 2, 4], dt.float32, tag="bqk")
            nc.sync.dma_start(bqk_sb[:], bqk_d[:])
            if masknz:
                emask_sb = cpool.tile([128, 16], dt.float32, tag="emask")
                nc.sync.dma_start(emask_sb[:], emask_d[:])

            x_sb = [None] * 4

            def load_x(tb):
                xt = bpool.tile([128, 8, 512], dt.bfloat16, tag=f"x{tb}",
                                name=f"x{tb}")
                nc.sync.dma_start(xt[:, 0:4, :], xT_d[tb, :, 0:4, :])
                nc.sync.dma_start(xt[:, 4:8, :], xT_d[tb, :, 4:8, :])
                x_sb[tb] = xt

            load_x(0)
            wq_sb = bpool.tile([128, 8, 512], dt.bfloat16, tag="wq")
            nc.sync.dma_start(wq_sb[:, :, 0:128], wqT_d[:, :, 0:128])
            wk_sb = bpool.tile([128, 8, 512], dt.bfloat16, tag="wk")
            nc.sync.dma_start(wk_sb[:, :, 0:128], wkT_d[:, :, 0:128])
            nc.sync.dma_start(wq_sb[:, :, 128:512], wqT_d[:, :, 128:512])
            nc.sync.dma_start(wk_sb[:, :, 128:512], wkT_d[:, :, 128:512])
            wv_sb = bpool.tile([128, 8, 512], dt.bfloat16, tag="wv")
            nc.sync.dma_start(wv_sb[:], wvT_d[:])
            for tb in range(1, 4):
                load_x(tb)

            qt = [[bpool.tile([128, 512], dt.bfloat16, tag=f"q{j}_{tb}",
                              name=f"q{j}_{tb}") for tb in range(4)]
                  for j in range(4)]
            kt = [[bpool.tile([128, 512], dt.bfloat16, tag=f"k{j}_{tb}",
                              name=f"k{j}_{tb}") for tb in range(4)]
                  for j in range(4)]
            v_sb = bpool.tile([128, 16, 8, 65], dt.bfloat16, tag="v")
            nc.gpsimd.memset(v_sb[:, :, :, 64:65], 1.0)
            e_sb = [bpool.tile([128, 34 * 512], dt.bfloat16, tag=f"e{s}",
                               name=f"e{s}") for s in range(2)]

            # ---- projections ------------------------------------------
            def proj_qk(j, tb, which, ps=None):
                ms = slice(j * 128, (j + 1) * 128)
                w = wq_sb if which == 0 else wk_sb
                dst = (qt if which == 0 else kt)[j][tb]
                if ps is None:
                    p = ps_proj.tile([128, 512], dt.float32, tag="proj",
                                     name="pqk")
                else:
                    p = ps
                for kc in range(8):
                    nc.tensor.matmul(p[:], w[:, kc, ms], x_sb[tb][:, kc, :],
                                     start=(kc == 0), stop=(kc == 7))
                nc.vector.tensor_scalar_add(dst[:], p[:],
                                            bqk_sb[:, which, j:j + 1])

            def proj_v(m):
                p = ps_proj.tile([128, 512], dt.float32, tag="proj", name="pv")
                msl = slice((m % 4) * 128, (m % 4 + 1) * 128)
                for kc in range(8):
                    nc.tensor.matmul(p[:], x_sb[m // 4][:, kc, msl],
                                     wv_sb[:, kc, :],
                                     start=(kc == 0), stop=(kc == 7))
                nc.vector.tensor_copy(
                    v_sb[:, m, :, 0:64],
                    p[:].rearrange("p (h d) -> p h d", h=8))

            # ---- attention --------------------------------------------
            def score_group(h, gi):
                j, p = h // 2, h % 2
                psl = slice(p * 64, p * 64 + 64)
                blocks = GROUPS[gi]
                ncols = 512 * GROUP_SIZES[gi]
                width = 1536 if gi % 2 == 0 else 1024
                sc = ps_sc.tile([128, width], dt.float32,
                                tag=f"sc{gi % 2}", name=f"sc{gi % 2}")
                for (sb, c, t0, w, off) in blocks:
                    lhs = kt[j][sb // 4][psl,
                                         (sb % 4) * 128:(sb % 4) * 128 + 128]
                    rhs = qt[j][c][psl, t0 - c * 512: t0 - c * 512 + w]
                    nc.tensor.matmul(sc[:, off:off + w], lhs, rhs,
                                     start=True, stop=True)
                eoff = sum(GROUP_SIZES[:gi]) * 512
                nc.scalar.activation(e_sb[h % 2][:, eoff:eoff + ncols],
                                     sc[:, 0:ncols], AF.Exp, scale=0.125)
                for (sb, c, t0, w, off) in blocks:
                    boff = eoff + off
                    if masknz:
                        nc.gpsimd.tensor_scalar_mul(
                            e_sb[h % 2][:, boff:boff + w],
                            e_sb[h % 2][:, boff:boff + w],
                            emask_sb[:, sb:sb + 1])
                    if c == sb // 4:        # diagonal block: tri mask
                        nc.vector.tensor_mul(
                            e_sb[h % 2][:, boff:boff + 128],
                            e_sb[h % 2][:, boff:boff + 128], tri_sb[:])

            def pv_batch(h, g, po):
                for (m, s2, first, last) in PV_PLAN[g]:
                    c = m // 4
                    gi_b, ecol = BLOCK_POS[(s2, c)]
                    t0 = max(c * 512, s2 * 128)
                    off = ecol + m * 128 - t0
                    pm = po[m % 2]
                    nc.tensor.matmul(pm[:],
                                     e_sb[h % 2][:, off:off + 128],
                                     v_sb[:, s2, h, :],
                                     start=first, stop=last,
                                     skip_group_check=True)
                    if last:
                        rz = spool.tile([128, 1], dt.float32, tag="rz",
                                        name="rz")
                        nc.vector.reciprocal(rz[:], pm[:, 64:65])
                        ot = spool.tile([128, 64], dt.float32, tag="ot",
                                        name="ot")
                        nc.vector.tensor_scalar_mul(ot[:], pm[:, 0:64],
                                                    rz[:])
                        nc.sync.dma_start(
                            out_d[m * 128:(m + 1) * 128,
                                  h * 64:(h + 1) * 64], ot[:])

            # ---- global emission schedule -----------------------------
            # fillers: (deadline (head, group) or None, closure)
            fillers = []

            def F(fn, dl=None):
                fillers.append((dl, fn))

            # head-0 k pieces: kt[0][tb] first used at sb=4tb
            first_group_of_sb = {}
            for gi, blocks in enumerate(GROUPS):
                for (sb, c, t0, w, off) in blocks:
                    first_group_of_sb.setdefault(sb, gi)
            F(lambda: proj_qk(0, 3, 0), (0, 1))             # qt[0][3] at c3
            for tb in (1, 2, 3):
                F(lambda tb=tb: proj_qk(0, tb, 1),
                  (0, first_group_of_sb[4 * tb]))
            for m in range(16):
                F(lambda m=m: proj_v(m),
                  (0, min(V_DEADLINE.get(m, N_GROUPS) - 1, N_GROUPS - 1)))
            for j in (1, 2, 3):
                for tb in range(4):
                    F(lambda j=j, tb=tb: proj_qk(j, tb, 0),
                      (2 * j, 0 if tb < 3 else 1))
                for tb in range(4):
                    F(lambda j=j, tb=tb: proj_qk(j, tb, 1),
                      (2 * j, first_group_of_sb[4 * tb]))

            def drain_fillers(now, budget):
                done = 0
                rest = []
                for (dl, fn) in fillers:
                    if dl is not None and dl <= now:
                        fn()
                        done += 1
                    else:
                        rest.append((dl, fn))
                fillers[:] = rest
                while done < budget and fillers:
                    _, fn = fillers.pop(0)
                    fn()
                    done += 1

            # startup: minimum pieces for head-0 group 0 (sb0: c0,c1,c2),
            # pipelined across proj + borrowed score-tile PSUM banks
            bootA = ps_sc.tile([128, 1536], dt.float32, tag="sc0", name="sc0")
            bootB = ps_sc.tile([128, 1024], dt.float32, tag="sc1", name="sc1")
            proj_qk(0, 0, 0)
            proj_qk(0, 0, 1, ps=bootA[:, 0:512])
            proj_qk(0, 1, 0, ps=bootB[:, 0:512])
            proj_qk(0, 2, 0, ps=bootA[:, 512:1024])

            for h in range(8):
                po = [ps_pv.tile([128, 65], dt.float32, tag=f"po{s}",
                                 name=f"po{s}") for s in range(2)]
                for gi in range(N_GROUPS):
                    budget = 1 if (h < 2 or gi % 3 == 0) else 0
                    drain_fillers((h, gi), budget)
                    score_group(h, gi)
                    if gi + 1 <= N_GROUPS - 1:
                        pv_batch(h, gi + 1, po)
                pv_batch(h, N_GROUPS, po)
            assert not fillers, f"{len(fillers)} fillers left"

    nc.compile()
    return nc


def _prep_core_inputs(c, x, mask, WqF, bq, Wk, bk, WvF):
    b, half = divmod(c, 2)
    hs = half * CH

    xT = np.ascontiguousarray(x[b].T.astype(BF16))  # [1024, 2048]
    xTd = np.ascontiguousarray(xT.reshape(8, 128, 4, 512).transpose(2, 1, 0, 3))

    def wT(W):
        Ws = W[hs:hs + CH]
        return np.ascontiguousarray(
            Ws.T.astype(BF16).reshape(8, 128, 512).transpose(1, 0, 2))

    bqk = np.ascontiguousarray(
        np.stack([bq[hs:hs + CH].reshape(4, 128).T,
                  bk[hs:hs + CH].reshape(4, 128).T], axis=1).astype(np.float32))

    tri = np.triu(np.ones((128, 128), BF16))

    d = {"xT": xTd, "wqT": wT(WqF), "wkT": wT(Wk), "wvT": wT(WvF),
         "bqk": bqk, "tri": tri}
    if (mask != 0).any():
        em = np.exp(mask[b, 0, 0]).reshape(16, 128).T.astype(np.float32)
        d["emask"] = np.ascontiguousarray(em)
    return d


def _run(inputs, trace=False, trace_kwargs=None):
    from concourse.bass_utils import run_bass_kernel_spmd

    args = {k: np.asarray(v) for k, v in inputs.items()}
    WqF = (args["Wq"].astype(np.float64)
           + LORA_SCALE * (args["qB"].astype(np.float64)
                           @ args["qA"].astype(np.float64))).astype(np.float32)
    WvF = (args["Wv"].astype(np.float64)
           + LORA_SCALE * (args["vB"].astype(np.float64)
                           @ args["vA"].astype(np.float64))).astype(np.float32)
    mask = args["attention_mask"]
    masknz = bool((mask != 0).any())

    in_maps = [
        _prep_core_inputs(c, args["hidden_states"], mask,
                          WqF, args["bq"], args["Wk"], args["bk"], WvF)
        for c in range(N_CORES)
    ]

    key = f"nc{int(masknz)}"
    if key not in _cached:
        _cached[key] = _build_nc(masknz)
    nc = _cached[key]

    res = run_bass_kernel_spmd(
        nc, in_maps, core_ids=list(range(N_CORES)), trace=trace,
        **(trace_kwargs or {}),
    )

    bv = args["bv"]
    full = np.empty((B, T, H), np.float32)
    for c in range(N_CORES):
        b, half = divmod(c, 2)
        hs = half * CH
        full[b, :, hs:hs + CH] = res.results[c]["out"] + bv[hs:hs + CH]
    return full, res


def kernel(**inputs):
    full, _ = _run(inputs, trace=False)
    return full
